# revision 29
# baseline (speedup 1.0000x reference)
"""Trainium2 Bass kernel for nn_Attention3D (GroupNorm + channel-attention + proj + residual).

Sharding: the spatial axis N = d*h*w = 32768 is split across 8 cores (Nc=4096
per core, both batch elements on every core). ONE AllReduce of [128, 776]:
per batch, a full-width [G0 | S0] block (258 cols) plus the diagonal half
[G1_diag | S1] (130 cols) of the symmetric Gram G = X_s X_s^T; the missing
corner G[1-block, 0:128] is reconstructed post-AR by one PE transpose.

Key algebra (validated against the reference in numpy):
  - Channel-attention logits contract over N, so
        L_b = A G_b B^T + (A S) w^T + u (B S)^T + N u w^T
    with A = Wq diag(alpha), B = Wk diag(alpha), u = Wq beta + bq,
    w = Wk beta + bk. Only [G | S] needs the network; q/k are never
    materialized, which deletes the 47us q/k projection pass.
  - GroupNorm mean/var derive from the SAME payload (mean from S, E[x^2]
    from diag G), so nothing upstream of the AllReduce touches the [c,n]
    copy of x -- the Gram front half and the post-AR back half decouple,
    and unrolled bodies software-pipeline (body r's Gram + AllReduce are
    emitted before body r-1's post-AR compute, hiding AR latency under
    PE work with no deadlock).
  - softmax(attn) @ v followed by proj collapses into a per-batch weight
    G_b' = P blockdiag(attn) (Wv diag(alpha)) applied directly to raw x,
    with a per-batch bias vector carrying all bias/affine terms.
  - x is staged in BOTH layouts ([c,n] for pass 2 + residual, [n,c] tiles
    with a baked ones-column for the Gram pass); the host transpose is
    free. Pass 2 adds bias+residual in-place in PSUM and DMAs straight
    to DRAM, so no output SBUF staging is needed.
"""
import sys

sys.path.insert(0, "/opt/trn_rl_repo")

import numpy as np
import concourse.bass as bass
import concourse.tile as tile
from concourse import mybir
from concourse.bass_utils import run_bass_kernel_spmd

F32 = mybir.dt.float32
F32R = mybir.dt.float32r
BF16 = mybir.dt.bfloat16
ALU = mybir.AluOpType
ACT = mybir.ActivationFunctionType

S = 8            # cores
B, C = 2, 256
N = 32 * 32 * 32
Nc = N // S      # 4096 spatial positions per core
H, HD = 4, 64
G = 8            # groupnorm groups
EPS = 1e-5
SM_SCALE = float(HD) ** -0.5
NT = Nc // 128   # 32 [n,c] tiles per batch
TW = C + 2       # xt tile width incl. ones column (+zero pad: even moving dim)
CW1 = 130        # ci=1 Gram block: diagonal half (128) + S + pad (G symmetric)
BW = TW + CW1    # per-batch AllReduce block stride
CCW = 2 * BW     # AllReduce payload width


def _split_excess_waits(nc, max_waits=1):
    """This container's walrus rejects >1 sem wait per instruction; move the
    overflow onto same-engine NoOps inserted immediately before."""
    ctr = 0
    for bb in nc.cur_f.blocks:
        insts = bb.instructions
        i = 0
        while i < len(insts):
            ins = insts[i]
            si = ins.sync_info
            if si is not None and len(si.on_wait) > max_waits:
                waits = list(si.on_wait)
                si.on_wait = waits[:max_waits]
                overflow = waits[max_waits:]
                pos = i
                for j in range(0, len(overflow), max_waits):
                    ctr += 1
                    nop = mybir.InstNoOp(name=f"I-ws-{ctr}", ins=[], outs=[])
                    nop.engine = ins.engine
                    nop.sync_info = mybir.SyncInfo(
                        on_wait=overflow[j : j + max_waits], on_update=[]
                    )
                    insts.insert(pos, nop)
                    pos += 1
                    i += 1
            i += 1


def build_nc(split_waits=True, loop_r=None, upto=99, unroll_r=None,
             no_ar=False, no_reload=False):
    """loop_r=None builds the real kernel. loop_r=R builds a timing variant:
    the collective runs once up-front, then the compute body repeats R times
    inside a hardware For_i loop. upto (timing variant only): emit only
    loop-body phases <= upto: 0=x reload, 2=gram+ccdma, 3=post-AR prep,
    4=logits, 5=softmax, 6=fused weights, 7=pass2+out.
    unroll_r=R: the FULL body (input DMAs, Gram, AllReduce, logits, softmax,
    pass 2, output DMAs) emitted R times, software-pipelined (collectives
    inside a HW For_i desync the mesh). Slope between two R values =
    per-invocation HW time including the collective."""
    nc = bass.Bass(num_devices=S)

    xs_d = nc.declare_dram_parameter("xs", [2 * B, 128, Nc], BF16, isOutput=False)
    xt_d = nc.declare_dram_parameter("xt", [B, 128, NT * TW], BF16, isOutput=False)
    wtqk_d = nc.declare_dram_parameter("wtqk", [C, 512], F32R, isOutput=False)
    wv_d = nc.declare_dram_parameter("wv", [C, C], F32R, isOutput=False)
    pt_d = nc.declare_dram_parameter("pt", [C, C], F32R, isOutput=False)
    gnw_d = nc.declare_dram_parameter("gnw", [C, 1], F32, isOutput=False)
    gnb_d = nc.declare_dram_parameter("gnb", [C, 1], F32, isOutput=False)
    bqk_d = nc.declare_dram_parameter("bqk", [1, 512], F32R, isOutput=False)
    bv_d = nc.declare_dram_parameter("bv", [C, 1], F32R, isOutput=False)
    pb_d = nc.declare_dram_parameter("pb", [1, C], F32, isOutput=False)
    g4_d = nc.declare_dram_parameter("g4", [128, 4], F32, isOutput=False)
    e4_d = nc.declare_dram_parameter("e4", [4, 128], F32, isOutput=False)
    const_d = nc.declare_dram_parameter("konst", [128, 257], F32R, isOutput=False)
    dmask_d = nc.declare_dram_parameter("dmask", [128, CCW], F32, isOutput=False)
    ident_d = nc.declare_dram_parameter("ident", [128, 128], F32R, isOutput=False)
    out_d = nc.declare_dram_parameter("out", [2 * B, 128, Nc], BF16, isOutput=True)

    cci = nc.dram_tensor("cci", [128, CCW], F32R)
    cco = nc.dram_tensor("cco", [128, CCW], F32R, addr_space="Shared")
    rg = [list(range(S))]

    with tile.TileContext(nc) as tc:
        with (
            tc.tile_pool(name="big", bufs=1) as big,        # resident x (both layouts)
            tc.tile_pool(name="wpool", bufs=1) as wpool,    # weights & per-batch mats
            tc.tile_pool(name="small", bufs=1) as small,    # stats / vectors
            tc.tile_pool(name="ochunk", bufs=8) as ochunk,  # pass-2 output staging
            tc.tile_pool(name="p_g", bufs=1, space="PSUM") as p_g,
            tc.tile_pool(name="p_work", bufs=3, space="PSUM") as p_work,
            tc.tile_pool(name="p_misc", bufs=2, space="PSUM") as p_misc,
        ):
            # ---------- one-time loads ----------
            x_sb = []  # t = b*2+cb -> [128, Nc] channel-major
            for t in range(4):
                xt_ = big.tile([128, Nc], BF16, tag=f"x{t}", name=f"x{t}")
                eng = nc.sync if t < 2 else nc.scalar
                eng.dma_start(out=xt_[:], in_=xs_d[t])
                x_sb.append(xt_)
            xt_sb = []  # b -> [128, NT*TW] spatial-major tiles (+ones col)
            for b in range(B):
                xt_ = big.tile([128, NT * TW], BF16, tag=f"xt{b}", name=f"xt{b}")
                nc.scalar.dma_start(out=xt_[:], in_=xt_d[b])
                xt_sb.append(xt_)
            wtqk_sb = []
            for k in range(2):
                w = wpool.tile([128, 512], F32R, tag=f"wtqk{k}", name=f"wtqk{k}")
                nc.scalar.dma_start(out=w[:], in_=wtqk_d[k * 128:(k + 1) * 128, :])
                wtqk_sb.append(w)
            wv_sb, pt_sb = [], []
            for k in range(2):
                w = wpool.tile([128, C], F32R, tag=f"wv{k}", name=f"wv{k}")
                nc.sync.dma_start(out=w[:], in_=wv_d[k * 128:(k + 1) * 128, :])
                wv_sb.append(w)
                p = wpool.tile([128, C], F32R, tag=f"pt{k}", name=f"pt{k}")
                nc.sync.dma_start(out=p[:], in_=pt_d[k * 128:(k + 1) * 128, :])
                pt_sb.append(p)
            gnw_sb, gnb_sb, bv_sb = [], [], []
            for k in range(2):
                sl = slice(k * 128, (k + 1) * 128)
                gw = small.tile([128, 1], F32, tag=f"gnw{k}", name=f"gnw{k}")
                nc.sync.dma_start(out=gw[:], in_=gnw_d[sl, :])
                gnw_sb.append(gw)
                gb = small.tile([128, 1], F32, tag=f"gnb{k}", name=f"gnb{k}")
                nc.sync.dma_start(out=gb[:], in_=gnb_d[sl, :])
                gnb_sb.append(gb)
                bv = small.tile([128, 1], F32R, tag=f"bv{k}", name=f"bv{k}")
                nc.sync.dma_start(out=bv[:], in_=bv_d[sl, :])
                bv_sb.append(bv)

            pb_sb = small.tile([1, C], F32, tag="pb", name="pb")
            nc.sync.dma_start(out=pb_sb[:], in_=pb_d[:])
            bqk_sb = small.tile([1, 512], F32R, tag="bqk", name="bqk")
            nc.sync.dma_start(out=bqk_sb[:], in_=bqk_d[:])
            g4_sb = small.tile([128, 4], F32, tag="g4", name="g4")
            nc.sync.dma_start(out=g4_sb[:], in_=g4_d[:])
            e4_sb = small.tile([4, 128], F32, tag="e4", name="e4")
            nc.sync.dma_start(out=e4_sb[:], in_=e4_d[:])
            dmask4_sb = wpool.tile([128, CCW], F32, tag="dmask", name="dmask")
            nc.sync.dma_start(out=dmask4_sb[:], in_=dmask_d[:])
            ident_sb = wpool.tile([128, 128], F32R, tag="ident", name="ident")
            nc.sync.dma_start(out=ident_sb[:], in_=ident_d[:])

            eps41 = small.tile([4, 1], F32, tag="eps", name="eps")
            nc.gpsimd.memset(eps41[:], EPS)
            konst_sb = wpool.tile([128, 257], F32R, tag="konst", name="konst")
            nc.sync.dma_start(out=konst_sb[:], in_=const_d[:])
            one11 = konst_sb[0:1, 256:257]
            scr41 = small.tile([4, 1], F32, tag="scr", name="scr")
            # preload the sqrt activation table while DMAs run
            nc.scalar.activation(out=scr41[:], in_=eps41[:], func=ACT.Sqrt)

            def emit_front(reload_xt):
                """Gram blocks [G_b | S_b] -> cci, then the AllReduce.
                Generator: yields after each (b,ci) chunk so the previous
                body's post-AR chain can interleave into the PE stream."""
                if reload_xt and not no_reload:
                    for b in range(B):
                        nc.scalar.dma_start(out=xt_sb[b][:], in_=xt_d[b])
                g1s = p_g.tile([128, 2 * CW1], F32, tag="g1s", name="g1s")
                for b in range(B):
                    for ci in range(2):
                        t = b * 2 + ci
                        w = TW if ci == 0 else CW1
                        roff = 0 if ci == 0 else 128  # rhs col offset within tile
                        if ci == 0:
                            gps = p_g.tile([128, w], F32, tag=f"g{b}{ci}", name=f"g{b}{ci}")[:]
                        else:
                            gps = g1s[:, b * CW1:(b + 1) * CW1]  # two 520B blocks share a bank
                        for k in range(NT):
                            nc.tensor.matmul(
                                gps,
                                xt_sb[b][:, k * TW + ci * 128: k * TW + ci * 128 + 128],
                                xt_sb[b][:, k * TW + roff:(k + 1) * TW],
                                start=(k == 0), stop=(k == NT - 1),
                                skip_group_check=(ci == 1),
                            )
                            if k == NT // 2 - 1:
                                yield
                        gcp = small.tile([128, w], F32R, tag=f"gcp{t}", name=f"gcp{t}")
                        if ci == 0:
                            nc.vector.tensor_copy(gcp[:], gps)
                        else:
                            nc.scalar.copy(out=gcp[:], in_=gps)
                        # gpsimd stream: a trigger waiting on Gram results
                        # stalls nothing (its next op, the AllReduce, needs
                        # them anyway) -- on Act it would stall softmax
                        nc.gpsimd.dma_start(
                            out=cci[:, b * BW + ci * TW: b * BW + ci * TW + w], in_=gcp[:]
                        )
                        yield
                if not no_ar:
                    nc.gpsimd.collective_compute(
                        "AllReduce", ALU.add, replica_groups=rg, ins=[cci[:]], outs=[cco[:]]
                    )

            def emit_back(reload_xs, upto=99):
                """post-AllReduce: stats chain, then per-batch chains (logits,
                softmax, fused weights, pass 2) so batch 1's scalar chain
                overlaps batch 0's pass-2 matmuls; then next body's reloads."""
                gg = wpool.tile([128, CCW], F32R, tag="gg", name="gg")
                nc.gpsimd.dma_start(out=gg[:], in_=cco[:])

                # ----- group stats from [G|S]: mean from S, E[x^2] from diag G -----
                st2x = small.tile([128, 8], F32, tag="st2x", name="st2x")
                for t in range(4):
                    b, ci = t // 2, t % 2
                    sc = b * BW + ci * TW + (C if ci == 0 else 128)
                    nc.vector.tensor_copy(st2x[:, t:t + 1], gg[:, sc:sc + 1])
                dga = small.tile([128, CCW], F32, tag="dga", name="dga")
                nc.vector.tensor_mul(dga[:], gg[:], dmask4_sb[:])
                for t in range(4):
                    b, ci = t // 2, t % 2
                    go = b * BW + ci * TW
                    gw = C if ci == 0 else 128
                    nc.vector.reduce_sum(
                        out=st2x[:, 4 + t:5 + t], in_=dga[:, go:go + gw],
                        axis=mybir.AxisListType.X,
                    )
                # missing Gram corner G[1-block, 0:128] = (G[0-block, 128:256])^T
                gt_sb = []
                for b in range(B):
                    ptp = p_misc.tile([128, 128], F32R, tag="m", name="ptp")
                    nc.tensor.transpose(ptp[:], gg[:, b * BW + 128: b * BW + 256], ident_sb[:])
                    gt = wpool.tile([128, 128], F32R, tag=f"gt{b}", name=f"gt{b}")
                    nc.vector.tensor_copy(gt[:], ptp[:])
                    gt_sb.append(gt)
                psum_g = p_misc.tile([4, 8], F32, tag="m", name="psum_g")
                nc.tensor.matmul(psum_g[:], g4_sb[:], st2x[:], start=True, stop=True)
                gsb = small.tile([4, 8], F32, tag="gsb", name="gsb")
                nc.vector.tensor_copy(gsb[:], psum_g[:])
                var44 = small.tile([4, 4], F32, tag="var44", name="var44")
                # mean^2 on the scalar engine straight from PSUM: runs in
                # parallel with the gsb copy instead of waiting for it
                nc.scalar.activation(out=var44[:], in_=psum_g[:, 0:4], func=ACT.Square)
                nc.vector.tensor_sub(var44[:], gsb[:, 4:8], var44[:])
                rstd44 = small.tile([4, 4], F32, tag="rstd44", name="rstd44")
                nc.scalar.activation(
                    out=rstd44[:], in_=var44[:], func=ACT.Sqrt, bias=eps41[:], scale=1.0
                )
                nc.vector.reciprocal(out=rstd44[:], in_=rstd44[:])
                # preload the exp table right after the last sqrt
                nc.scalar.activation(out=scr41[:], in_=rstd44[:, 0:1], func=ACT.Exp)
                yield

                for b in range(B):
                    # ----- per-batch affine prep -----
                    a_b, bb_b, wts_b = [], [], []
                    for cb in range(2):
                        t = b * 2 + cb
                        pmean = p_misc.tile([128, 1], F32, tag="m", name="pmean")
                        nc.tensor.matmul(
                            pmean[:], e4_sb[:], gsb[:, t:t + 1], start=True, stop=True
                        )
                        prstd = p_misc.tile([128, 1], F32, tag="m", name="prstd")
                        nc.tensor.matmul(
                            prstd[:], e4_sb[:], rstd44[:, t:t + 1], start=True, stop=True
                        )
                        a = small.tile([128, 1], F32, tag=f"a{t}", name=f"a{t}")
                        nc.vector.tensor_mul(a[:], prstd[:], gnw_sb[cb][:])
                        w = wpool.tile([128, 512], F32R, tag=f"wts{t}", name=f"wts{t}")
                        nc.vector.tensor_scalar_mul(out=w[:], in0=wtqk_sb[cb][:], scalar1=a[:])
                        na = small.tile([128, 1], F32, tag=f"na{t}", name=f"na{t}")
                        nc.scalar.mul(out=na[:], in_=a[:], mul=-1.0)
                        bbv = small.tile([128, 1], F32R, tag=f"bb{t}", name=f"bb{t}")
                        nc.vector.scalar_tensor_tensor(
                            out=bbv[:], in0=pmean[:], scalar=na[:], in1=gnb_sb[cb][:],
                            op0=ALU.mult, op1=ALU.add,
                        )  # gnb - mean*a
                        a_b.append(a); bb_b.append(bbv); wts_b.append(w)
                        yield

                    # rowbias rb (u|w), colsums sg (AS|BS), rank-1 stacks
                    prb = p_misc.tile([1, 512], F32, tag="m", name="prb")
                    nc.tensor.matmul(prb[:], bb_b[0][:], wtqk_sb[0][:], start=True, stop=False)
                    nc.tensor.matmul(prb[:], bb_b[1][:], wtqk_sb[1][:], start=False, stop=False)
                    nc.tensor.matmul(prb[:], one11, bqk_sb[:], start=False, stop=True)
                    rb = small.tile([1, 512], F32, tag=f"rb{b}", name=f"rb{b}")
                    nc.vector.tensor_copy(rb[:], prb[:])
                    psg = p_misc.tile([1, 512], F32, tag="m", name="psg")
                    nc.tensor.matmul(psg[:], gg[:, b * BW + C:b * BW + C + 1],
                                     wts_b[0][:], start=True, stop=False)
                    nc.tensor.matmul(psg[:], gg[:, b * BW + TW + 128:b * BW + TW + 129],
                                     wts_b[1][:], start=False, stop=True)
                    sg = small.tile([1, 512], F32, tag=f"sg{b}", name=f"sg{b}")
                    nc.vector.tensor_copy(sg[:], psg[:])
                    rbn = small.tile([1, 512], F32, tag=f"rbn{b}", name=f"rbn{b}")
                    nc.scalar.mul(out=rbn[:], in_=rb[:], mul=float(N))
                    lq = small.tile([3, 256], F32, tag=f"lq{b}", name=f"lq{b}")
                    nc.sync.dma_start(out=lq[0:1, :], in_=rb[0:1, 0:256])
                    nc.sync.dma_start(out=lq[1:2, :], in_=sg[0:1, 0:256])
                    nc.sync.dma_start(out=lq[2:3, :], in_=rbn[0:1, 0:256])
                    rk = small.tile([3, 256], F32, tag=f"rk{b}", name=f"rk{b}")
                    nc.sync.dma_start(out=rk[0:1, :], in_=sg[0:1, 256:512])
                    nc.sync.dma_start(out=rk[1:2, :], in_=rb[0:1, 256:512])
                    nc.sync.dma_start(out=rk[2:3, :], in_=rb[0:1, 256:512])
                    yield

                    if upto < 4:
                        continue
                    # ----- logits: M = G (diag(a) Wk^T) ; L = Wq_a^T M + rank-1 -----
                    att_b = [None, None]
                    msb = wpool.tile([128, 2 * C], F32R, tag=f"msb{b}", name=f"msb{b}")
                    for ei in range(2):
                        mps = p_misc.tile([128, C], F32, tag="m", name="mps")
                        for ci in range(2):
                            if ci == 0:
                                lhsT = gg[:, b * BW + ei * 128: b * BW + ei * 128 + 128]
                            elif ei == 0:
                                lhsT = gt_sb[b][:]      # reconstructed corner
                            else:
                                lhsT = gg[:, b * BW + TW: b * BW + TW + 128]
                            nc.tensor.matmul(
                                mps[:], lhsT, wts_b[ci][:, 256:512],
                                start=(ci == 0), stop=(ci == 1),
                            )
                        if ei == 0:
                            nc.vector.tensor_copy(msb[:, ei * C:(ei + 1) * C], mps[:])
                        else:
                            nc.scalar.copy(out=msb[:, ei * C:(ei + 1) * C], in_=mps[:])
                        yield
                    for ci in range(2):
                        lps = p_misc.tile([128, C], F32, tag="m", name="lps")
                        for ei in range(2):
                            nc.tensor.matmul(
                                lps[:],
                                wts_b[ei][:, ci * 128: ci * 128 + 128],
                                msb[:, ei * C:(ei + 1) * C],
                                start=(ei == 0), stop=False,
                            )
                        nc.tensor.matmul(
                            lps[:],
                            lq[:, ci * 128: ci * 128 + 128],
                            rk[:],
                            start=False, stop=True, skip_group_check=True,
                        )
                        if upto < 5:
                            continue
                        # ----- extract head-diagonal blocks + softmax -----
                        atc = small.tile([128, 64], F32, tag=f"atc{ci}", name=f"atc{ci}")
                        nc.vector.tensor_copy(atc[0:64, :], lps[0:64, ci * 128: ci * 128 + 64])
                        nc.vector.tensor_copy(atc[64:128, :], lps[64:128, ci * 128 + 64: ci * 128 + 128])
                        negm = small.tile([128, 1], F32, tag=f"negm{ci}", name=f"negm{ci}")
                        nc.vector.reduce_max(
                            out=negm[:], in_=atc[:], axis=mybir.AxisListType.X, negate=True
                        )
                        nc.scalar.mul(out=negm[:], in_=negm[:], mul=SM_SCALE)
                        esb = small.tile([128, 64], F32, tag=f"esb{ci}", name=f"esb{ci}")
                        nc.scalar.activation(
                            out=esb[:], in_=atc[:], func=ACT.Exp,
                            bias=negm[:], scale=SM_SCALE,
                        )
                        ssum = small.tile([128, 1], F32, tag=f"ssum{ci}", name=f"ssum{ci}")
                        nc.vector.reduce_sum(out=ssum[:], in_=esb[:], axis=mybir.AxisListType.X)
                        nc.vector.reciprocal(out=ssum[:], in_=ssum[:])
                        sm = small.tile([128, 64], F32, tag=f"sm{b}{ci}", name=f"sm{b}{ci}")
                        nc.vector.tensor_scalar_mul(out=sm[:], in0=esb[:], scalar1=ssum[:])
                        att_b[ci] = sm
                        yield

                    if upto < 6:
                        continue
                    # ----- blockdiag + fused per-batch weights -----
                    ablk = []
                    for k in range(2):
                        ab = wpool.tile([128, 256], F32R, tag=f"ablk{b}{k}", name=f"ablk{b}{k}")
                        nc.vector.tensor_copy(ab[:], konst_sb[:, 0:256])
                        h0, h1 = 2 * k, 2 * k + 1
                        nc.vector.tensor_copy(ab[0:64, h0 * 64:(h0 + 1) * 64], att_b[k][0:64, :])
                        nc.vector.tensor_copy(ab[64:128, h1 * 64:(h1 + 1) * 64], att_b[k][64:128, :])
                        ablk.append(ab)
                    mbt_b, gbt_b = [], []
                    for m in range(2):
                        pm = p_misc.tile([128, 256], F32, tag="m", name="pm")
                        msl = slice(m * 128, (m + 1) * 128)
                        nc.tensor.matmul(pm[:], ablk[0][:, msl], pt_sb[0][:], start=True, stop=False)
                        nc.tensor.matmul(pm[:], ablk[1][:, msl], pt_sb[1][:], start=False, stop=True)
                        mbt = wpool.tile([128, 256], F32R, tag=f"mbt{b}{m}", name=f"mbt{b}{m}")
                        if m == 0:
                            nc.vector.tensor_copy(mbt[:], pm[:])
                        else:
                            nc.scalar.copy(out=mbt[:], in_=pm[:])
                        mbt_b.append(mbt)
                        yield
                    for g in range(2):
                        pg2 = p_misc.tile([128, 256], F32, tag="m", name="pg2")
                        gsl = slice(g * 128, (g + 1) * 128)
                        nc.tensor.matmul(pg2[:], wv_sb[0][:, gsl], mbt_b[0][:], start=True, stop=False)
                        nc.tensor.matmul(pg2[:], wv_sb[1][:, gsl], mbt_b[1][:], start=False, stop=True)
                        gbt = wpool.tile([128, 256], F32R, tag=f"gbt{b}{g}", name=f"gbt{b}{g}")
                        if g == 0:
                            nc.vector.tensor_copy(gbt[:], pg2[:])
                        else:
                            nc.scalar.copy(out=gbt[:], in_=pg2[:])
                        gbt_b.append(gbt)
                    pbeta = p_misc.tile([1, C], F32, tag="m", name="pbeta")
                    nc.tensor.matmul(pbeta[:], bb_b[0][:], gbt_b[0][:], start=True, stop=False)
                    nc.tensor.matmul(pbeta[:], bb_b[1][:], gbt_b[1][:], start=False, stop=False)
                    nc.tensor.matmul(pbeta[:], bv_sb[0][:], mbt_b[0][:], start=False, stop=False)
                    nc.tensor.matmul(pbeta[:], bv_sb[1][:], mbt_b[1][:], start=False, stop=True)
                    brow = small.tile([1, C], F32, tag=f"brow{b}", name=f"brow{b}")
                    nc.vector.tensor_add(brow[:], pbeta[:], pb_sb[:])
                    beta_b = []
                    for mo in range(2):
                        bet = small.tile([128, 1], F32, tag=f"beta{b}{mo}", name=f"beta{b}{mo}")
                        nc.sync.dma_start(out=bet[:], in_=brow[0:1, mo * 128:(mo + 1) * 128])
                        beta_b.append(bet)
                    # fold the GroupNorm scale into G_b (after the bias matmuls
                    # read the f32r original); bf16 copy feeds pass 2
                    gbf_b = []
                    for g in range(2):
                        gbf = wpool.tile([128, 256], BF16, tag=f"gbf{b}{g}", name=f"gbf{b}{g}")
                        nc.vector.tensor_scalar_mul(
                            out=gbf[:], in0=gbt_b[g][:], scalar1=a_b[g][:]
                        )
                        gbf_b.append(gbf)
                    yield

                    if upto < 7:
                        continue
                    # ----- pass 2 (this batch): out = G_b' x + beta + x -----
                    for mo in range(2):
                        t = b * 2 + mo
                        msl = slice(mo * 128, (mo + 1) * 128)
                        for nt in range(Nc // 512):
                            nsl = slice(nt * 512, (nt + 1) * 512)
                            po = p_work.tile([128, 512], F32, tag="w", name="po")
                            nc.tensor.matmul(po[:], gbf_b[0][:, msl], x_sb[b * 2][:, nsl],
                                             start=True, stop=False)
                            nc.tensor.matmul(po[:], gbf_b[1][:, msl], x_sb[b * 2 + 1][:, nsl],
                                             start=False, stop=True)
                            osb = ochunk.tile([128, 512], BF16, tag="o", name="osb")
                            nc.vector.scalar_tensor_tensor(
                                out=osb[:], in0=po[:], scalar=beta_b[mo][:],
                                in1=x_sb[t][:, nsl], op0=ALU.add, op1=ALU.add,
                            )
                            nc.sync.dma_start(out=out_d[t][:, nsl], in_=osb[:])
                        yield
                    if reload_xs and upto >= 7 and not no_reload:
                        # this batch's [c,n] tiles are dead now; prefetch next body's
                        for cb in range(2):
                            t = b * 2 + cb
                            eng = nc.sync if cb == 0 else nc.scalar
                            eng.dma_start(out=x_sb[t][:], in_=xs_d[t])

                if reload_xs and upto >= 7 and not no_reload:
                    for k in range(2):
                        nc.sync.dma_start(out=wtqk_sb[k][:], in_=wtqk_d[k * 128:(k + 1) * 128, :])
                        nc.sync.dma_start(out=wv_sb[k][:], in_=wv_d[k * 128:(k + 1) * 128, :])
                        nc.sync.dma_start(out=pt_sb[k][:], in_=pt_d[k * 128:(k + 1) * 128, :])

            def drain(g):
                for _ in g:
                    pass

            def zip_emit(fg, bg):
                while True:
                    advanced = False
                    try:
                        next(fg); advanced = True
                    except StopIteration:
                        pass
                    try:
                        next(bg); advanced = True
                    except StopIteration:
                        pass
                    if not advanced:
                        return

            if loop_r is None:
                R = unroll_r or 1
                drain(emit_front(reload_xt=False))
                for r in range(1, R):
                    zip_emit(emit_front(reload_xt=True), emit_back(reload_xs=True))
                drain(emit_back(reload_xs=False))
            else:
                # timing variant: collective once, compute body looped
                drain(emit_front(reload_xt=False))
                with tc.For_i(0, loop_r, 1):
                    for t in range(4):
                        nc.sync.dma_start(out=x_sb[t][:], in_=xs_d[t])
                    for b in range(B):
                        nc.sync.dma_start(out=xt_sb[b][:], in_=xt_d[b])
                    if upto >= 2:
                        for b in range(B):
                            for ci in range(2):
                                t = b * 2 + ci
                                gps = p_g.tile([128, TW], F32, tag=f"g{b}{ci}", name=f"lg{b}{ci}")
                                for k in range(NT):
                                    nc.tensor.matmul(
                                        gps[:],
                                        xt_sb[b][:, k * TW + ci * 128: k * TW + ci * 128 + 128],
                                        xt_sb[b][:, k * TW:(k + 1) * TW],
                                        start=(k == 0), stop=(k == NT - 1),
                                    )
                                nc.sync.dma_start(out=cci[:, t * TW:(t + 1) * TW], in_=gps[:])
                    if upto >= 3:
                        drain(emit_back(reload_xs=False, upto=upto))

    if split_waits:
        _split_excess_waits(nc)
    return nc


_NC_CACHE = None


def _get_nc():
    global _NC_CACHE
    if _NC_CACHE is None:
        _NC_CACHE = build_nc()
    return _NC_CACHE


def _prep_inputs(x, gn_w, gn_b, qkv_w, qkv_b, proj_w, proj_b):
    x = np.ascontiguousarray(np.asarray(x, np.float32)).reshape(B, C, N)
    qkv_w = np.asarray(qkv_w, np.float32)
    qkv_b = np.asarray(qkv_b, np.float32)
    proj_w = np.asarray(proj_w, np.float32)
    shared = {
        "wtqk": np.ascontiguousarray(qkv_w[0:512].T),
        "wv": np.ascontiguousarray(qkv_w[512:768]),
        "pt": np.ascontiguousarray(proj_w.T),
        "gnw": np.asarray(gn_w, np.float32).reshape(C, 1),
        "gnb": np.asarray(gn_b, np.float32).reshape(C, 1),
        "bqk": qkv_b[0:512].reshape(1, 512),
        "bv": qkv_b[512:768].reshape(C, 1),
        "pb": np.asarray(proj_b, np.float32).reshape(1, C),
    }
    g4 = np.zeros((128, 4), np.float32)
    for p in range(128):
        g4[p, p // 32] = 1.0 / (32.0 * N)
    e4 = np.zeros((4, 128), np.float32)
    for p in range(128):
        e4[p // 32, p] = 1.0
    shared["g4"] = g4
    shared["e4"] = e4
    konst = np.zeros((128, 257), np.float32)
    konst[0, 256] = 1.0
    shared["konst"] = konst
    dmask = np.zeros((128, 2 * BW), np.float32)
    for p in range(128):
        for t in range(4):
            b, ci = t // 2, t % 2
            dmask[p, b * BW + ci * TW + p] = 1.0
    shared["dmask"] = dmask
    shared["ident"] = np.eye(128, dtype=np.float32)
    import ml_dtypes
    bf = ml_dtypes.bfloat16
    in_maps = []
    for s in range(S):
        xsh = x[:, :, s * Nc:(s + 1) * Nc]                      # [B, C, Nc]
        xs = np.ascontiguousarray(xsh).reshape(2 * B, 128, Nc).astype(bf)
        # [n,c] tiles + ones column: xt[b][p, k*TW + c] = xsh[b, c, k*128 + p]
        xt4 = xsh.transpose(0, 2, 1).reshape(B, NT, 128, C).transpose(0, 2, 1, 3)
        pad = np.zeros((B, 128, NT, 2), np.float32)
        pad[:, :, :, 0] = 1.0
        xt = np.concatenate([xt4, pad], axis=3).reshape(B, 128, NT * TW).astype(bf)
        in_maps.append({"xs": xs, "xt": np.ascontiguousarray(xt), **{k: v for k, v in shared.items()}})
    return in_maps


def kernel(x, gn_w, gn_b, qkv_w, qkv_b, proj_w, proj_b):
    nc = _get_nc()
    in_maps = _prep_inputs(x, gn_w, gn_b, qkv_w, qkv_b, proj_w, proj_b)
    res = run_bass_kernel_spmd(nc, in_maps, list(range(S)), trace=False)
    shards = [np.asarray(res.results[s]["out"], np.float32).reshape(B, C, Nc) for s in range(S)]
    return np.concatenate(shards, axis=2).reshape(B, C, 32, 32, 32).astype(np.float32)
